# revision 8
# baseline (speedup 1.0000x reference)
"""Trainium2 Bass kernel for nn_BiRNNImputerModel (bidirectional GRU imputer).

Strategy (v3):
  - 8 cores: cores 0-3 forward GRU, cores 4-7 backward GRU (time-reversed
    inputs), data-parallel over batch within a direction (NB=32 per core).
  - Transposed on-chip layout [feature/H, batch]; all matmuls bf16
    (fp8/DoubleRow measured slower: LDWEIGHTS of 256 fp8 columns costs 127ns
    vs 27ns issue for bf16 tiles).  Weights scaled x64 (pow2-exact);
    descale rides the sigmoid/tanh `scale` port.
  - Gate math fused across all 4 H-folds ([128,128] ops); biases injected
    into PSUM via two K=8 indicator matmuls (the bank's start=True).
  - h_new = t3 + zh with t3 = n*(1-z), zh = z*h.  The readout matmul is
    split Wro^T*zh + Wro^T*t3 so it completes without waiting for h_new,
    shortening the xhat->x_in feedback path.
  - x_in kept in 8-step blocks (one mask DMA per block, mask rows are
    partitions 64:127); outputs staged 4 steps per DMA; readout biases
    (bro/bout) applied on the host.

PSUM: one [128,512] fp32 bank per step parity: cols 0:128 r, 128:256 z,
256:384 gi_n, 384:512 gh_n; plus a [128,32] readout tile per parity.
"""

import os
import sys

for _p in ("/opt/trn_rl_repo", "/root/.axon_site/_ro/trn_rl_repo"):
    if os.path.isdir(_p) and _p not in sys.path:
        sys.path.insert(0, _p)

import numpy as np
import ml_dtypes

import concourse.bass as bass
import concourse.tile as tile
from concourse import mybir
from concourse.bass_utils import run_bass_kernel_spmd

BF16 = ml_dtypes.bfloat16

B, S, N, C = 128, 512, 64, 1
F = N * C          # 64
H = 512
NB = 32            # batch per core (128 / 4)
NFOLD = 4          # H / 128
WS = 64.0          # weight scale (pow2)
AF = mybir.ActivationFunctionType
ALU = mybir.AluOpType

XBLK = 8           # steps per x/mask prefetch block
OBLK = 4           # steps per output store block


def _legalize_multiwait(nc, max_waits=1):
    """walrus in this image only encodes one sync-wait per instruction;
    hoist extra waits onto preceding NoOps."""
    n_fix = 0
    for f in nc.m.functions:
        for blk in f.blocks:
            new = []
            for ins in blk.instructions:
                si = getattr(ins, "sync_info", None)
                if si is not None and si.on_wait and len(si.on_wait) > max_waits:
                    waits = list(si.on_wait)
                    si.on_wait = waits[-max_waits:]
                    for i, w in enumerate(waits[:-max_waits]):
                        new.append(
                            mybir.InstNoOp(
                                name=f"{ins.name}-waitfix-{i}",
                                engine=ins.engine,
                                sync_info=mybir.SyncInfo(on_wait=[w], on_update=[]),
                                bass_nofuse=True,
                            )
                        )
                        n_fix += 1
                new.append(ins)
            blk.instructions[:] = new
    return n_fix


def build_nc(n_steps):
    nc = bass.Bass()
    dt = mybir.dt

    xm = nc.dram_tensor("xm", [F, n_steps, 2 * NB], dt.bfloat16, kind="ExternalInput")
    whh = nc.dram_tensor("whh", [128, 48 * 128], dt.bfloat16, kind="ExternalInput")
    wih = nc.dram_tensor("wih", [128, 12 * 128], dt.bfloat16, kind="ExternalInput")
    wro = nc.dram_tensor("wro", [128, 4 * 128], dt.bfloat16, kind="ExternalInput")
    btr = nc.dram_tensor("btr", [4, 128], dt.bfloat16, kind="ExternalInput")
    btz = nc.dram_tensor("btz", [4, 128], dt.bfloat16, kind="ExternalInput")
    btn = nc.dram_tensor("btn", [8, 128], dt.bfloat16, kind="ExternalInput")
    indt = nc.dram_tensor("indt", [8, 256], dt.bfloat16, kind="ExternalInput")
    bro = nc.dram_tensor("bro", [F, 1], dt.float32, kind="ExternalInput")

    op_out = nc.dram_tensor("op", [128, n_steps, NB], dt.bfloat16, kind="ExternalOutput")

    with tile.TileContext(nc) as tc:
        with (
            tc.tile_pool(name="singles", bufs=1) as singles,
            tc.tile_pool(name="hist", bufs=1) as hist,
            tc.tile_pool(name="xblk", bufs=2) as xblkp,
            tc.tile_pool(name="xin", bufs=2) as xinp,
            tc.tile_pool(name="oblk", bufs=2) as oblkp,
            tc.tile_pool(name="work", bufs=2) as work,
            tc.tile_pool(name="ps", bufs=2, space="PSUM") as psp,
            tc.tile_pool(name="psro", bufs=2, space="PSUM") as psrop,
        ):
            whh_sb = singles.tile([128, 48, 128], dt.bfloat16)
            nc.sync.dma_start(out=whh_sb, in_=whh[:])
            wih_sb = singles.tile([128, 12, 128], dt.bfloat16)
            nc.sync.dma_start(out=wih_sb, in_=wih[:])
            wro_sb = singles.tile([128, 4, 128], dt.bfloat16)
            nc.sync.dma_start(out=wro_sb, in_=wro[:])
            btr_sb = singles.tile([4, 128], dt.bfloat16)
            nc.sync.dma_start(out=btr_sb, in_=btr[:])
            btz_sb = singles.tile([4, 128], dt.bfloat16)
            nc.sync.dma_start(out=btz_sb, in_=btz[:])
            btn_sb = singles.tile([8, 128], dt.bfloat16)
            nc.sync.dma_start(out=btn_sb, in_=btn[:])
            ind_sb = singles.tile([8, 256], dt.bfloat16)
            nc.sync.dma_start(out=ind_sb, in_=indt[:])
            bro_sb = singles.tile([F, 1], dt.float32)
            nc.sync.dma_start(out=bro_sb, in_=bro[:])

            hb = hist.tile([128, NFOLD, 2, NB], dt.bfloat16)
            nc.vector.memset(hb[:, :, 0, :], 0.0)

            def load_xmb(t0):
                n = min(XBLK, n_steps + 1 - t0)
                xmb = xblkp.tile([F, XBLK, 2 * NB], dt.bfloat16, tag="xmb",
                                 name=f"xmb{t0}")
                nc.sync.dma_start(out=xmb[:, 0:n, :], in_=xm[:, t0 - 1:t0 - 1 + n, :])
                return xmb

            def new_xin8(t0):
                n = min(XBLK, n_steps + 1 - t0)
                x8 = xinp.tile([128, XBLK, NB], dt.bfloat16, tag="xin8",
                               name=f"xin8_{t0}")
                nc.sync.dma_start(out=x8[F:128, 0:n, :],
                                  in_=xm[:, t0 - 1:t0 - 1 + n, NB:2 * NB])
                return x8

            xmb_cur = load_xmb(1)
            xin_cur = new_xin8(1)
            xmb_nxt = xin_nxt = None

            # bootstrap xin(1): xhat_0 = bro, masked with x_0
            nc.vector.memset(xin_cur[0:F, 0, :], 0.0)
            nc.scalar.activation(out=xin_cur[0:F, 0, :], in_=xin_cur[0:F, 0, :],
                                 func=AF.Identity, bias=bro_sb, scale=1.0)
            nc.vector.copy_predicated(
                xin_cur[0:F, 0, :],
                xmb_cur[:, 0, NB:2 * NB].bitcast(mybir.dt.uint16),
                xmb_cur[:, 0, 0:NB])

            out_cur = oblkp.tile([128, OBLK, NB], dt.bfloat16, tag="out4", name="out0")

            def alloc_ps(t):
                ps_r = psp.tile([128, 128], dt.float32, tag="ps_r", name=f"psr{t}")
                ps_z = psp.tile([128, 128], dt.float32, tag="ps_z", name=f"psz{t}")
                ps_n = psp.tile([128, 256], dt.float32, tag="ps_n", name=f"psn{t}")
                nc.tensor.matmul(ps_r, btr_sb, ind_sb[0:4, 0:128],
                                 start=True, stop=False, skip_group_check=True)
                nc.tensor.matmul(ps_z, btz_sb, ind_sb[0:4, 0:128],
                                 start=True, stop=False, skip_group_check=True)
                nc.tensor.matmul(ps_n, btn_sb, ind_sb,
                                 start=True, stop=False, skip_group_check=True)
                return ps_r, ps_z, ps_n

            ps_cur = alloc_ps(1)

            for t in range(1, n_steps + 1):
                pv, cur = (t - 1) % 2, t % 2
                s = (t - 1) % XBLK           # xin slot for step t
                so = (t - 1) % OBLK          # out slot for out index t-1

                ps_r, ps_z, ps_n = ps_cur
                ps_ro = psrop.tile([128, NB], dt.float32, tag="ps_ro", name=f"ro{t}")
                if t < n_steps:
                    ps_nxt = alloc_ps(t + 1)   # bias matmuls run in tensor idle

                def dst(gate, m):
                    if gate == 0:
                        return ps_r[:, m * 32:m * 32 + 32]
                    if gate == 1:
                        return ps_z[:, m * 32:m * 32 + 32]
                    return ps_n[:, 128 + m * 32:128 + m * 32 + 32]

                def gh(gate, m, c2):
                    idx = gate * 16 + m * 4 + c2
                    nc.tensor.matmul(dst(gate, m), whh_sb[:, idx, :],
                                     hb[:, c2, pv, :],
                                     start=False, stop=False,
                                     skip_group_check=True)

                def gi(gate, m, stop=False):
                    tile_i = gate * 4 + m
                    off = ps_n[:, m * 32:m * 32 + 32] if gate == 2 else dst(gate, m)
                    nc.tensor.matmul(off, wih_sb[:, tile_i, :],
                                     xin_cur[:, s, :],
                                     start=False, stop=stop,
                                     skip_group_check=True)

                # r gate first (chain-critical), then n parts, then z
                for m in range(4):
                    for c2 in range(4):
                        gh(0, m, c2)
                for m in range(4):
                    gi(0, m, stop=(m == 3))
                r_t = work.tile([128, 128], dt.bfloat16, tag="r_t", name=f"r{t}")
                nc.scalar.activation(out=r_t, in_=ps_r, func=AF.Sigmoid,
                                     scale=1.0 / WS)
                for m in range(4):
                    for c2 in range(4):
                        gh(2, m, c2)
                for m in range(4):
                    gi(2, m, stop=(m == 3))
                for m in range(4):
                    for c2 in range(4):
                        gh(1, m, c2)
                for m in range(4):
                    gi(1, m, stop=(m == 3))
                z_t = work.tile([128, 128], dt.bfloat16, tag="z_t", name=f"z{t}")
                nc.scalar.activation(out=z_t, in_=ps_z, func=AF.Sigmoid,
                                     scale=1.0 / WS)
                zh_t = work.tile([128, 128], dt.bfloat16, tag="zh", name=f"zh{t}")
                nc.gpsimd.tensor_tensor(zh_t, z_t, hb[:, :, pv, :], ALU.mult)
                # chain: tmp -> nin -> tanh -> t3 -> h'
                tmp_t = work.tile([128, 128], dt.float32, tag="tmp", name=f"tm{t}")
                nc.vector.tensor_tensor(tmp_t, ps_n[:, 128:256], r_t, ALU.mult)
                nin_t = work.tile([128, 128], dt.float32, tag="nin", name=f"ni{t}")
                nc.vector.tensor_tensor(nin_t, ps_n[:, 0:128], tmp_t, ALU.add)
                omz_t = work.tile([128, 128], dt.bfloat16, tag="omz", name=f"om{t}")
                nc.vector.tensor_scalar(omz_t, z_t, -1.0, 1.0, ALU.mult, ALU.add)
                n_t = work.tile([128, 128], dt.bfloat16, tag="n_t", name=f"n{t}")
                nc.scalar.activation(out=n_t, in_=nin_t, func=AF.Tanh,
                                     scale=1.0 / WS)
                t3_t = work.tile([128, 128], dt.bfloat16, tag="t3", name=f"t3{t}")
                nc.vector.tensor_tensor(t3_t, n_t, omz_t, ALU.mult)
                # readout: Wro^T zh + Wro^T t3  (= Wro^T h_new)
                for k in range(4):
                    nc.tensor.matmul(ps_ro, wro_sb[:, k, :],
                                     zh_t[:, k * 32:(k + 1) * 32],
                                     start=(k == 0), stop=False,
                                     skip_group_check=True)
                for k in range(4):
                    nc.tensor.matmul(ps_ro, wro_sb[:, k, :],
                                     t3_t[:, k * 32:(k + 1) * 32],
                                     start=False, stop=(k == 3),
                                     skip_group_check=True)
                # h_new
                nc.vector.tensor_tensor(hb[:, :, cur, :], t3_t, zh_t, ALU.add)

                # stage output for out index t-1
                nc.scalar.activation(out=out_cur[:, so, :], in_=ps_ro, func=AF.Copy)
                if so == OBLK - 1 or t == n_steps:
                    nc.sync.dma_start(out=op_out[:, t - 1 - so:t, :],
                                      in_=out_cur[:, 0:so + 1, :])
                    if t < n_steps:
                        out_cur = oblkp.tile([128, OBLK, NB], dt.bfloat16,
                                             tag="out4", name=f"out{t}")

                # prefetch next x/mask block
                if s == 2:
                    t0n = t - s + XBLK
                    if t0n <= n_steps:
                        xmb_nxt = load_xmb(t0n)
                        xin_nxt = new_xin8(t0n)

                # x_in(t+1) feedback
                if t < n_steps:
                    s2 = t % XBLK
                    if s2 == 0:
                        xmb_cur, xin_cur = xmb_nxt, xin_nxt
                    nc.vector.tensor_scalar_add(xin_cur[0:F, s2, :],
                                                ps_ro[0:F, :], bro_sb)
                    nc.vector.copy_predicated(
                        xin_cur[0:F, s2, :],
                        xmb_cur[:, s2, NB:2 * NB].bitcast(mybir.dt.uint16),
                        xmb_cur[:, s2, 0:NB])
                    ps_cur = ps_nxt

    _legalize_multiwait(nc)
    return nc


_NC_CACHE = {}


def _get_nc(n_steps):
    if n_steps not in _NC_CACHE:
        _NC_CACHE[n_steps] = build_nc(n_steps)
    return _NC_CACHE[n_steps]


def _prep_core_inputs(x2d, m2d, Wih, Whh, bih, bhh, Wro, bro, Wout_half, n_steps):
    """Per-core input map. x2d/m2d: [NB, S_loc, F] float32/bool already
    direction-ordered (time-reversed for backward cores)."""
    xt = x2d[:, :n_steps].transpose(2, 1, 0)          # [F, t, NB]
    mt = m2d[:, :n_steps].transpose(2, 1, 0).astype(np.float32)
    xmv = np.concatenate([xt, mt], axis=2).astype(BF16)  # [F, t, 2*NB]

    WT = np.ascontiguousarray(Whh.T).astype(np.float32) * WS  # [512, 1536]
    whh_t = np.empty((128, 48, 128), np.float32)
    for g in range(3):
        for m in range(4):
            for c2 in range(4):
                idx = g * 16 + m * 4 + c2
                whh_t[:, idx, :] = WT[128 * c2:128 * (c2 + 1),
                                      512 * g + 128 * m:512 * g + 128 * (m + 1)]
    WI = np.ascontiguousarray(Wih.T).astype(np.float32) * WS  # [128, 1536]
    wih_t = WI.reshape(128, 12, 128)

    wro_f = Wro.T.reshape(NFOLD, 128, F)               # [4,128,64]
    wout_f = Wout_half.T.reshape(NFOLD, 128, F)
    wro_t = np.concatenate([wro_f, wout_f], axis=2).transpose(1, 0, 2)  # [128,4,128]

    bsum = (bih + bhh).astype(np.float32) * WS
    btr = np.empty((4, 128), np.float32)
    btz = np.empty((4, 128), np.float32)
    for k in range(4):
        btr[k] = bsum[128 * k:128 * (k + 1)]                   # r folds
        btz[k] = bsum[512 + 128 * k:512 + 128 * (k + 1)]       # z folds
    btn = np.empty((8, 128), np.float32)
    for k in range(4):
        btn[k] = bih.astype(np.float32)[1024 + 128 * k:1024 + 128 * (k + 1)] * WS
        btn[4 + k] = bhh.astype(np.float32)[1024 + 128 * k:1024 + 128 * (k + 1)] * WS
    ind = np.zeros((8, 256), np.float32)
    for k in range(4):
        ind[k, 32 * k:32 * (k + 1)] = 1.0
        ind[4 + k, 128 + 32 * k:128 + 32 * (k + 1)] = 1.0

    return {
        "xm": xmv,
        "whh": whh_t.astype(BF16).reshape(128, 48 * 128),
        "wih": wih_t.astype(BF16).reshape(128, 12 * 128),
        "wro": np.ascontiguousarray(wro_t).astype(BF16).reshape(128, 4 * 128),
        "btr": btr.astype(BF16), "btz": btz.astype(BF16),
        "btn": btn.astype(BF16),
        "indt": ind.astype(BF16),
        "bro": bro.reshape(F, 1).astype(np.float32),
    }


def run_device(inputs, s_len=S, trace=False):
    n_steps = s_len - 1
    nc = _get_nc(n_steps)

    x2d = np.asarray(inputs["x"], np.float32).reshape(B, S, F)[:, :s_len]
    m2d = np.asarray(inputs["mask"]).reshape(B, S, F)[:, :s_len]

    in_maps = []
    for core in range(8):
        g = core % 4
        bsl = slice(NB * g, NB * (g + 1))
        if core < 4:
            im = _prep_core_inputs(
                x2d[bsl], m2d[bsl], inputs["Wih_f"], inputs["Whh_f"],
                inputs["bih_f"], inputs["bhh_f"], inputs["Wro_f"], inputs["bro_f"],
                np.asarray(inputs["Wout"])[:, :H], n_steps,
            )
        else:
            im = _prep_core_inputs(
                x2d[bsl, ::-1], m2d[bsl, ::-1], inputs["Wih_b"], inputs["Whh_b"],
                inputs["bih_b"], inputs["bhh_b"], inputs["Wro_b"], inputs["bro_b"],
                np.asarray(inputs["Wout"])[:, H:], n_steps,
            )
        in_maps.append(im)

    return run_bass_kernel_spmd(nc, in_maps, core_ids=list(range(8)), trace=trace)


def assemble(inputs, res, s_len=S):
    """Host-side gather: combine per-core outputs into full reference outputs."""
    n_steps = s_len - 1
    bro_f = np.asarray(inputs["bro_f"], np.float32)
    bro_b = np.asarray(inputs["bro_b"], np.float32)
    bout = np.asarray(inputs["bout"], np.float32)

    xh_f = np.empty((B, s_len, F), np.float32)
    xh_b = np.empty((B, s_len, F), np.float32)
    x_hat = np.empty((B, s_len, F), np.float32)

    for g in range(4):
        bsl = slice(NB * g, NB * (g + 1))
        rf = np.asarray(res.results[g]["op"], np.float32)       # [128, t, NB]
        rb = np.asarray(res.results[g + 4]["op"], np.float32)
        xh_f[bsl, 1:] = rf[:F].transpose(2, 1, 0) + bro_f
        xh_f[bsl, 0] = bro_f
        xh_b[bsl, :n_steps] = rb[:F].transpose(2, 1, 0)[:, ::-1] + bro_b
        xh_b[bsl, n_steps] = bro_b
        pf = rf[F:].transpose(2, 1, 0)
        pb = rb[F:].transpose(2, 1, 0)[:, ::-1]
        x_hat[bsl, 1:] = pf
        x_hat[bsl, 0] = 0.0
        x_hat[bsl, :n_steps] += pb
        x_hat[bsl] += bout

    return (
        x_hat.reshape(B, s_len, N, C),
        xh_f.reshape(B, s_len, N, C),
        xh_b.reshape(B, s_len, N, C),
    )


def kernel(**inputs):
    res = run_device(inputs, s_len=S)
    return assemble(inputs, res, s_len=S)


# revision 9
# speedup vs baseline: 1.0003x; 1.0003x over previous
"""Trainium2 Bass kernel for nn_BiRNNImputerModel (bidirectional GRU imputer).

Strategy (v3):
  - 8 cores: cores 0-3 forward GRU, cores 4-7 backward GRU (time-reversed
    inputs), data-parallel over batch within a direction (NB=32 per core).
  - Transposed on-chip layout [feature/H, batch]; all matmuls bf16
    (fp8/DoubleRow measured slower: LDWEIGHTS of 256 fp8 columns costs 127ns
    vs 27ns issue for bf16 tiles).  Weights scaled x64 (pow2-exact);
    descale rides the sigmoid/tanh `scale` port.
  - Gate math fused across all 4 H-folds ([128,128] ops); biases injected
    into PSUM via two K=8 indicator matmuls (the bank's start=True).
  - h_new = t3 + zh with t3 = n*(1-z), zh = z*h.  The readout matmul is
    split Wro^T*zh + Wro^T*t3 so it completes without waiting for h_new,
    shortening the xhat->x_in feedback path.
  - x_in kept in 8-step blocks (one mask DMA per block, mask rows are
    partitions 64:127); outputs staged 4 steps per DMA; readout biases
    (bro/bout) applied on the host.

PSUM: one [128,512] fp32 bank per step parity: cols 0:128 r, 128:256 z,
256:384 gi_n, 384:512 gh_n; plus a [128,32] readout tile per parity.
"""

import os
import sys

for _p in ("/opt/trn_rl_repo", "/root/.axon_site/_ro/trn_rl_repo"):
    if os.path.isdir(_p) and _p not in sys.path:
        sys.path.insert(0, _p)

import numpy as np
import ml_dtypes

import concourse.bass as bass
import concourse.tile as tile
from concourse import mybir
from concourse.bass_utils import run_bass_kernel_spmd

BF16 = ml_dtypes.bfloat16

B, S, N, C = 128, 512, 64, 1
F = N * C          # 64
H = 512
NB = 32            # batch per core (128 / 4)
NFOLD = 4          # H / 128
WS = 64.0          # weight scale (pow2)
AF = mybir.ActivationFunctionType
ALU = mybir.AluOpType

XBLK = 8           # steps per x/mask prefetch block
OBLK = 4           # steps per output store block


def _legalize_multiwait(nc, max_waits=1):
    """walrus in this image only encodes one sync-wait per instruction;
    hoist extra waits onto preceding NoOps."""
    n_fix = 0
    for f in nc.m.functions:
        for blk in f.blocks:
            new = []
            for ins in blk.instructions:
                si = getattr(ins, "sync_info", None)
                if si is not None and si.on_wait and len(si.on_wait) > max_waits:
                    waits = list(si.on_wait)
                    si.on_wait = waits[-max_waits:]
                    for i, w in enumerate(waits[:-max_waits]):
                        new.append(
                            mybir.InstNoOp(
                                name=f"{ins.name}-waitfix-{i}",
                                engine=ins.engine,
                                sync_info=mybir.SyncInfo(on_wait=[w], on_update=[]),
                                bass_nofuse=True,
                            )
                        )
                        n_fix += 1
                new.append(ins)
            blk.instructions[:] = new
    return n_fix


def build_nc(n_steps):
    nc = bass.Bass()
    dt = mybir.dt

    xm = nc.dram_tensor("xm", [F, n_steps, 2 * NB], dt.bfloat16, kind="ExternalInput")
    whh = nc.dram_tensor("whh", [128, 48 * 128], dt.bfloat16, kind="ExternalInput")
    wih = nc.dram_tensor("wih", [128, 12 * 128], dt.bfloat16, kind="ExternalInput")
    wro = nc.dram_tensor("wro", [128, 4 * 128], dt.bfloat16, kind="ExternalInput")
    btr = nc.dram_tensor("btr", [4, 128], dt.bfloat16, kind="ExternalInput")
    btz = nc.dram_tensor("btz", [4, 128], dt.bfloat16, kind="ExternalInput")
    btn = nc.dram_tensor("btn", [8, 128], dt.bfloat16, kind="ExternalInput")
    indt = nc.dram_tensor("indt", [8, 256], dt.bfloat16, kind="ExternalInput")
    bro = nc.dram_tensor("bro", [F, 1], dt.float32, kind="ExternalInput")

    op_out = nc.dram_tensor("op", [128, n_steps, NB], dt.bfloat16, kind="ExternalOutput")

    with tile.TileContext(nc) as tc:
        with (
            tc.tile_pool(name="singles", bufs=1) as singles,
            tc.tile_pool(name="hist", bufs=1) as hist,
            tc.tile_pool(name="xblk", bufs=2) as xblkp,
            tc.tile_pool(name="xin", bufs=2) as xinp,
            tc.tile_pool(name="oblk", bufs=2) as oblkp,
            tc.tile_pool(name="work", bufs=2) as work,
            tc.tile_pool(name="ps", bufs=2, space="PSUM") as psp,
            tc.tile_pool(name="psro", bufs=2, space="PSUM") as psrop,
        ):
            whh_sb = singles.tile([128, 48, 128], dt.bfloat16)
            nc.sync.dma_start(out=whh_sb, in_=whh[:])
            wih_sb = singles.tile([128, 12, 128], dt.bfloat16)
            nc.sync.dma_start(out=wih_sb, in_=wih[:])
            wro_sb = singles.tile([128, 4, 128], dt.bfloat16)
            nc.sync.dma_start(out=wro_sb, in_=wro[:])
            btr_sb = singles.tile([4, 128], dt.bfloat16)
            nc.sync.dma_start(out=btr_sb, in_=btr[:])
            btz_sb = singles.tile([4, 128], dt.bfloat16)
            nc.sync.dma_start(out=btz_sb, in_=btz[:])
            btn_sb = singles.tile([8, 128], dt.bfloat16)
            nc.sync.dma_start(out=btn_sb, in_=btn[:])
            ind_sb = singles.tile([8, 256], dt.bfloat16)
            nc.sync.dma_start(out=ind_sb, in_=indt[:])
            bro_sb = singles.tile([F, 1], dt.float32)
            nc.sync.dma_start(out=bro_sb, in_=bro[:])

            hb = hist.tile([128, NFOLD, 2, NB], dt.bfloat16)
            nc.vector.memset(hb[:, :, 0, :], 0.0)

            def load_xmb(t0):
                n = min(XBLK, n_steps + 1 - t0)
                xmb = xblkp.tile([F, XBLK, 2 * NB], dt.bfloat16, tag="xmb",
                                 name=f"xmb{t0}")
                nc.sync.dma_start(out=xmb[:, 0:n, :], in_=xm[:, t0 - 1:t0 - 1 + n, :])
                return xmb

            def new_xin8(t0):
                n = min(XBLK, n_steps + 1 - t0)
                x8 = xinp.tile([128, XBLK, NB], dt.bfloat16, tag="xin8",
                               name=f"xin8_{t0}")
                nc.sync.dma_start(out=x8[F:128, 0:n, :],
                                  in_=xm[:, t0 - 1:t0 - 1 + n, NB:2 * NB])
                return x8

            xmb_cur = load_xmb(1)
            xin_cur = new_xin8(1)
            xmb_nxt = xin_nxt = None

            # bootstrap xin(1): xhat_0 = bro, masked with x_0
            nc.vector.memset(xin_cur[0:F, 0, :], 0.0)
            nc.scalar.activation(out=xin_cur[0:F, 0, :], in_=xin_cur[0:F, 0, :],
                                 func=AF.Identity, bias=bro_sb, scale=1.0)
            nc.vector.copy_predicated(
                xin_cur[0:F, 0, :],
                xmb_cur[:, 0, NB:2 * NB].bitcast(mybir.dt.uint16),
                xmb_cur[:, 0, 0:NB])

            out_cur = oblkp.tile([128, OBLK, NB], dt.bfloat16, tag="out4", name="out0")

            def alloc_ps(t):
                # full-bank tiles (2KB/partition) so r/z/n never share a PSUM
                # bank -- bank sharing serializes readers on unrelated writers
                ps_r = psp.tile([128, 512], dt.float32, tag="ps_r", name=f"psr{t}")[:, 0:128]
                ps_z = psp.tile([128, 512], dt.float32, tag="ps_z", name=f"psz{t}")[:, 0:128]
                ps_n = psp.tile([128, 512], dt.float32, tag="ps_n", name=f"psn{t}")[:, 0:256]
                nc.tensor.matmul(ps_r, btr_sb, ind_sb[0:4, 0:128],
                                 start=True, stop=False, skip_group_check=True)
                nc.tensor.matmul(ps_z, btz_sb, ind_sb[0:4, 0:128],
                                 start=True, stop=False, skip_group_check=True)
                nc.tensor.matmul(ps_n, btn_sb, ind_sb,
                                 start=True, stop=False, skip_group_check=True)
                return ps_r, ps_z, ps_n

            ps_cur = alloc_ps(1)

            for t in range(1, n_steps + 1):
                pv, cur = (t - 1) % 2, t % 2
                s = (t - 1) % XBLK           # xin slot for step t
                so = (t - 1) % OBLK          # out slot for out index t-1

                ps_r, ps_z, ps_n = ps_cur
                ps_ro = psrop.tile([128, 512], dt.float32, tag="ps_ro",
                                   name=f"ro{t}")[:, 0:NB]
                if t < n_steps:
                    ps_nxt = alloc_ps(t + 1)   # bias matmuls run in tensor idle

                def dst(gate, m):
                    if gate == 0:
                        return ps_r[:, m * 32:m * 32 + 32]
                    if gate == 1:
                        return ps_z[:, m * 32:m * 32 + 32]
                    return ps_n[:, 128 + m * 32:128 + m * 32 + 32]

                def gh(gate, m, c2):
                    idx = gate * 16 + m * 4 + c2
                    nc.tensor.matmul(dst(gate, m), whh_sb[:, idx, :],
                                     hb[:, c2, pv, :],
                                     start=False, stop=False,
                                     skip_group_check=True)

                def gi(gate, m, stop=False):
                    tile_i = gate * 4 + m
                    off = ps_n[:, m * 32:m * 32 + 32] if gate == 2 else dst(gate, m)
                    nc.tensor.matmul(off, wih_sb[:, tile_i, :],
                                     xin_cur[:, s, :],
                                     start=False, stop=stop,
                                     skip_group_check=True)

                # r gate first (chain-critical), then n parts, then z
                for m in range(4):
                    for c2 in range(4):
                        gh(0, m, c2)
                for m in range(4):
                    gi(0, m, stop=(m == 3))
                r_t = work.tile([128, 128], dt.bfloat16, tag="r_t", name=f"r{t}")
                nc.scalar.activation(out=r_t, in_=ps_r, func=AF.Sigmoid,
                                     scale=1.0 / WS)
                for m in range(4):
                    for c2 in range(4):
                        gh(2, m, c2)
                for m in range(4):
                    gi(2, m, stop=(m == 3))
                for m in range(4):
                    for c2 in range(4):
                        gh(1, m, c2)
                for m in range(4):
                    gi(1, m, stop=(m == 3))
                z_t = work.tile([128, 128], dt.bfloat16, tag="z_t", name=f"z{t}")
                nc.scalar.activation(out=z_t, in_=ps_z, func=AF.Sigmoid,
                                     scale=1.0 / WS)
                zh_t = work.tile([128, 128], dt.bfloat16, tag="zh", name=f"zh{t}")
                nc.gpsimd.tensor_tensor(zh_t, z_t, hb[:, :, pv, :], ALU.mult)
                # chain: tmp -> nin -> tanh -> t3 -> h'
                tmp_t = work.tile([128, 128], dt.float32, tag="tmp", name=f"tm{t}")
                nc.vector.tensor_tensor(tmp_t, ps_n[:, 128:256], r_t, ALU.mult)
                nin_t = work.tile([128, 128], dt.float32, tag="nin", name=f"ni{t}")
                nc.vector.tensor_tensor(nin_t, ps_n[:, 0:128], tmp_t, ALU.add)
                omz_t = work.tile([128, 128], dt.bfloat16, tag="omz", name=f"om{t}")
                nc.vector.tensor_scalar(omz_t, z_t, -1.0, 1.0, ALU.mult, ALU.add)
                n_t = work.tile([128, 128], dt.bfloat16, tag="n_t", name=f"n{t}")
                nc.scalar.activation(out=n_t, in_=nin_t, func=AF.Tanh,
                                     scale=1.0 / WS)
                t3_t = work.tile([128, 128], dt.bfloat16, tag="t3", name=f"t3{t}")
                nc.vector.tensor_tensor(t3_t, n_t, omz_t, ALU.mult)
                # readout: Wro^T zh + Wro^T t3  (= Wro^T h_new)
                for k in range(4):
                    nc.tensor.matmul(ps_ro, wro_sb[:, k, :],
                                     zh_t[:, k * 32:(k + 1) * 32],
                                     start=(k == 0), stop=False,
                                     skip_group_check=True)
                for k in range(4):
                    nc.tensor.matmul(ps_ro, wro_sb[:, k, :],
                                     t3_t[:, k * 32:(k + 1) * 32],
                                     start=False, stop=(k == 3),
                                     skip_group_check=True)
                # h_new
                nc.vector.tensor_tensor(hb[:, :, cur, :], t3_t, zh_t, ALU.add)

                # stage output for out index t-1
                nc.scalar.activation(out=out_cur[:, so, :], in_=ps_ro, func=AF.Copy)
                if so == OBLK - 1 or t == n_steps:
                    nc.sync.dma_start(out=op_out[:, t - 1 - so:t, :],
                                      in_=out_cur[:, 0:so + 1, :])
                    if t < n_steps:
                        out_cur = oblkp.tile([128, OBLK, NB], dt.bfloat16,
                                             tag="out4", name=f"out{t}")

                # prefetch next x/mask block
                if s == 2:
                    t0n = t - s + XBLK
                    if t0n <= n_steps:
                        xmb_nxt = load_xmb(t0n)
                        xin_nxt = new_xin8(t0n)

                # x_in(t+1) feedback
                if t < n_steps:
                    s2 = t % XBLK
                    if s2 == 0:
                        xmb_cur, xin_cur = xmb_nxt, xin_nxt
                    nc.vector.tensor_scalar_add(xin_cur[0:F, s2, :],
                                                ps_ro[0:F, :], bro_sb)
                    nc.vector.copy_predicated(
                        xin_cur[0:F, s2, :],
                        xmb_cur[:, s2, NB:2 * NB].bitcast(mybir.dt.uint16),
                        xmb_cur[:, s2, 0:NB])
                    ps_cur = ps_nxt

    _legalize_multiwait(nc)
    return nc


_NC_CACHE = {}


def _get_nc(n_steps):
    if n_steps not in _NC_CACHE:
        _NC_CACHE[n_steps] = build_nc(n_steps)
    return _NC_CACHE[n_steps]


def _prep_core_inputs(x2d, m2d, Wih, Whh, bih, bhh, Wro, bro, Wout_half, n_steps):
    """Per-core input map. x2d/m2d: [NB, S_loc, F] float32/bool already
    direction-ordered (time-reversed for backward cores)."""
    xt = x2d[:, :n_steps].transpose(2, 1, 0)          # [F, t, NB]
    mt = m2d[:, :n_steps].transpose(2, 1, 0).astype(np.float32)
    xmv = np.concatenate([xt, mt], axis=2).astype(BF16)  # [F, t, 2*NB]

    WT = np.ascontiguousarray(Whh.T).astype(np.float32) * WS  # [512, 1536]
    whh_t = np.empty((128, 48, 128), np.float32)
    for g in range(3):
        for m in range(4):
            for c2 in range(4):
                idx = g * 16 + m * 4 + c2
                whh_t[:, idx, :] = WT[128 * c2:128 * (c2 + 1),
                                      512 * g + 128 * m:512 * g + 128 * (m + 1)]
    WI = np.ascontiguousarray(Wih.T).astype(np.float32) * WS  # [128, 1536]
    wih_t = WI.reshape(128, 12, 128)

    wro_f = Wro.T.reshape(NFOLD, 128, F)               # [4,128,64]
    wout_f = Wout_half.T.reshape(NFOLD, 128, F)
    wro_t = np.concatenate([wro_f, wout_f], axis=2).transpose(1, 0, 2)  # [128,4,128]

    bsum = (bih + bhh).astype(np.float32) * WS
    btr = np.empty((4, 128), np.float32)
    btz = np.empty((4, 128), np.float32)
    for k in range(4):
        btr[k] = bsum[128 * k:128 * (k + 1)]                   # r folds
        btz[k] = bsum[512 + 128 * k:512 + 128 * (k + 1)]       # z folds
    btn = np.empty((8, 128), np.float32)
    for k in range(4):
        btn[k] = bih.astype(np.float32)[1024 + 128 * k:1024 + 128 * (k + 1)] * WS
        btn[4 + k] = bhh.astype(np.float32)[1024 + 128 * k:1024 + 128 * (k + 1)] * WS
    ind = np.zeros((8, 256), np.float32)
    for k in range(4):
        ind[k, 32 * k:32 * (k + 1)] = 1.0
        ind[4 + k, 128 + 32 * k:128 + 32 * (k + 1)] = 1.0

    return {
        "xm": xmv,
        "whh": whh_t.astype(BF16).reshape(128, 48 * 128),
        "wih": wih_t.astype(BF16).reshape(128, 12 * 128),
        "wro": np.ascontiguousarray(wro_t).astype(BF16).reshape(128, 4 * 128),
        "btr": btr.astype(BF16), "btz": btz.astype(BF16),
        "btn": btn.astype(BF16),
        "indt": ind.astype(BF16),
        "bro": bro.reshape(F, 1).astype(np.float32),
    }


def run_device(inputs, s_len=S, trace=False):
    n_steps = s_len - 1
    nc = _get_nc(n_steps)

    x2d = np.asarray(inputs["x"], np.float32).reshape(B, S, F)[:, :s_len]
    m2d = np.asarray(inputs["mask"]).reshape(B, S, F)[:, :s_len]

    in_maps = []
    for core in range(8):
        g = core % 4
        bsl = slice(NB * g, NB * (g + 1))
        if core < 4:
            im = _prep_core_inputs(
                x2d[bsl], m2d[bsl], inputs["Wih_f"], inputs["Whh_f"],
                inputs["bih_f"], inputs["bhh_f"], inputs["Wro_f"], inputs["bro_f"],
                np.asarray(inputs["Wout"])[:, :H], n_steps,
            )
        else:
            im = _prep_core_inputs(
                x2d[bsl, ::-1], m2d[bsl, ::-1], inputs["Wih_b"], inputs["Whh_b"],
                inputs["bih_b"], inputs["bhh_b"], inputs["Wro_b"], inputs["bro_b"],
                np.asarray(inputs["Wout"])[:, H:], n_steps,
            )
        in_maps.append(im)

    return run_bass_kernel_spmd(nc, in_maps, core_ids=list(range(8)), trace=trace)


def assemble(inputs, res, s_len=S):
    """Host-side gather: combine per-core outputs into full reference outputs."""
    n_steps = s_len - 1
    bro_f = np.asarray(inputs["bro_f"], np.float32)
    bro_b = np.asarray(inputs["bro_b"], np.float32)
    bout = np.asarray(inputs["bout"], np.float32)

    xh_f = np.empty((B, s_len, F), np.float32)
    xh_b = np.empty((B, s_len, F), np.float32)
    x_hat = np.empty((B, s_len, F), np.float32)

    for g in range(4):
        bsl = slice(NB * g, NB * (g + 1))
        rf = np.asarray(res.results[g]["op"], np.float32)       # [128, t, NB]
        rb = np.asarray(res.results[g + 4]["op"], np.float32)
        xh_f[bsl, 1:] = rf[:F].transpose(2, 1, 0) + bro_f
        xh_f[bsl, 0] = bro_f
        xh_b[bsl, :n_steps] = rb[:F].transpose(2, 1, 0)[:, ::-1] + bro_b
        xh_b[bsl, n_steps] = bro_b
        pf = rf[F:].transpose(2, 1, 0)
        pb = rb[F:].transpose(2, 1, 0)[:, ::-1]
        x_hat[bsl, 1:] = pf
        x_hat[bsl, 0] = 0.0
        x_hat[bsl, :n_steps] += pb
        x_hat[bsl] += bout

    return (
        x_hat.reshape(B, s_len, N, C),
        xh_f.reshape(B, s_len, N, C),
        xh_b.reshape(B, s_len, N, C),
    )


def kernel(**inputs):
    res = run_device(inputs, s_len=S)
    return assemble(inputs, res, s_len=S)


# revision 10
# speedup vs baseline: 1.0005x; 1.0001x over previous
"""Trainium2 Bass kernel for nn_BiRNNImputerModel (bidirectional GRU imputer).

Strategy (v3):
  - 8 cores: cores 0-3 forward GRU, cores 4-7 backward GRU (time-reversed
    inputs), data-parallel over batch within a direction (NB=32 per core).
  - Transposed on-chip layout [feature/H, batch]; all matmuls bf16
    (fp8/DoubleRow measured slower: LDWEIGHTS of 256 fp8 columns costs 127ns
    vs 27ns issue for bf16 tiles).  Weights scaled x64 (pow2-exact);
    descale rides the sigmoid/tanh `scale` port.
  - Gate math fused across all 4 H-folds ([128,128] ops); biases injected
    into PSUM via two K=8 indicator matmuls (the bank's start=True).
  - h_new = t3 + zh with t3 = n*(1-z), zh = z*h.  The readout matmul is
    split Wro^T*zh + Wro^T*t3 so it completes without waiting for h_new,
    shortening the xhat->x_in feedback path.
  - x_in kept in 8-step blocks (one mask DMA per block, mask rows are
    partitions 64:127); outputs staged 4 steps per DMA; readout biases
    (bro/bout) applied on the host.

PSUM: one [128,512] fp32 bank per step parity: cols 0:128 r, 128:256 z,
256:384 gi_n, 384:512 gh_n; plus a [128,32] readout tile per parity.
"""

import os
import sys

for _p in ("/opt/trn_rl_repo", "/root/.axon_site/_ro/trn_rl_repo"):
    if os.path.isdir(_p) and _p not in sys.path:
        sys.path.insert(0, _p)

import numpy as np
import ml_dtypes

import concourse.bass as bass
import concourse.tile as tile
from concourse import mybir
from concourse.bass_utils import run_bass_kernel_spmd

BF16 = ml_dtypes.bfloat16

B, S, N, C = 128, 512, 64, 1
F = N * C          # 64
H = 512
NB = 32            # batch per core (128 / 4)
NFOLD = 4          # H / 128
WS = 64.0          # weight scale (pow2)
AF = mybir.ActivationFunctionType
ALU = mybir.AluOpType

XBLK = 8           # steps per x/mask prefetch block
OBLK = 4           # steps per output store block


def _legalize_multiwait(nc, max_waits=1):
    """walrus in this image only encodes one sync-wait per instruction;
    hoist extra waits onto preceding NoOps."""
    n_fix = 0
    for f in nc.m.functions:
        for blk in f.blocks:
            new = []
            for ins in blk.instructions:
                si = getattr(ins, "sync_info", None)
                if si is not None and si.on_wait and len(si.on_wait) > max_waits:
                    waits = list(si.on_wait)
                    si.on_wait = waits[-max_waits:]
                    for i, w in enumerate(waits[:-max_waits]):
                        new.append(
                            mybir.InstNoOp(
                                name=f"{ins.name}-waitfix-{i}",
                                engine=ins.engine,
                                sync_info=mybir.SyncInfo(on_wait=[w], on_update=[]),
                                bass_nofuse=True,
                            )
                        )
                        n_fix += 1
                new.append(ins)
            blk.instructions[:] = new
    return n_fix


def build_nc(n_steps):
    nc = bass.Bass()
    dt = mybir.dt

    xm = nc.dram_tensor("xm", [F, n_steps, 2 * NB], dt.bfloat16, kind="ExternalInput")
    whh = nc.dram_tensor("whh", [128, 48 * 128], dt.bfloat16, kind="ExternalInput")
    wih = nc.dram_tensor("wih", [128, 12 * 128], dt.bfloat16, kind="ExternalInput")
    wro = nc.dram_tensor("wro", [128, 4 * 128], dt.bfloat16, kind="ExternalInput")
    btr = nc.dram_tensor("btr", [4, 128], dt.bfloat16, kind="ExternalInput")
    btz = nc.dram_tensor("btz", [4, 128], dt.bfloat16, kind="ExternalInput")
    btn = nc.dram_tensor("btn", [8, 128], dt.bfloat16, kind="ExternalInput")
    indt = nc.dram_tensor("indt", [8, 256], dt.bfloat16, kind="ExternalInput")
    bro = nc.dram_tensor("bro", [F, 1], dt.float32, kind="ExternalInput")

    op_out = nc.dram_tensor("op", [128, n_steps, NB], dt.bfloat16, kind="ExternalOutput")

    with tile.TileContext(nc) as tc:
        with (
            tc.tile_pool(name="singles", bufs=1) as singles,
            tc.tile_pool(name="hist", bufs=1) as hist,
            tc.tile_pool(name="xblk", bufs=2) as xblkp,
            tc.tile_pool(name="xin", bufs=2) as xinp,
            tc.tile_pool(name="oblk", bufs=2) as oblkp,
            tc.tile_pool(name="work", bufs=2) as work,
            tc.tile_pool(name="ps", bufs=2, space="PSUM") as psp,
            tc.tile_pool(name="psro", bufs=2, space="PSUM") as psrop,
        ):
            whh_sb = singles.tile([128, 48, 128], dt.bfloat16)
            nc.sync.dma_start(out=whh_sb, in_=whh[:])
            wih_sb = singles.tile([128, 12, 128], dt.bfloat16)
            nc.sync.dma_start(out=wih_sb, in_=wih[:])
            wro_sb = singles.tile([128, 4, 128], dt.bfloat16)
            nc.sync.dma_start(out=wro_sb, in_=wro[:])
            btr_sb = singles.tile([4, 128], dt.bfloat16)
            nc.sync.dma_start(out=btr_sb, in_=btr[:])
            btz_sb = singles.tile([4, 128], dt.bfloat16)
            nc.sync.dma_start(out=btz_sb, in_=btz[:])
            btn_sb = singles.tile([8, 128], dt.bfloat16)
            nc.sync.dma_start(out=btn_sb, in_=btn[:])
            ind_sb = singles.tile([8, 256], dt.bfloat16)
            nc.sync.dma_start(out=ind_sb, in_=indt[:])
            bro_sb = singles.tile([F, 1], dt.float32)
            nc.sync.dma_start(out=bro_sb, in_=bro[:])

            hb = hist.tile([128, NFOLD, 2, NB], dt.bfloat16)
            nc.vector.memset(hb[:, :, 0, :], 0.0)

            def load_xmb(t0):
                n = min(XBLK, n_steps + 1 - t0)
                xmb = xblkp.tile([F, XBLK, 2 * NB], dt.bfloat16, tag="xmb",
                                 name=f"xmb{t0}")
                nc.sync.dma_start(out=xmb[:, 0:n, :], in_=xm[:, t0 - 1:t0 - 1 + n, :])
                return xmb

            def new_xin8(t0):
                n = min(XBLK, n_steps + 1 - t0)
                x8 = xinp.tile([128, XBLK, NB], dt.bfloat16, tag="xin8",
                               name=f"xin8_{t0}")
                nc.sync.dma_start(out=x8[F:128, 0:n, :],
                                  in_=xm[:, t0 - 1:t0 - 1 + n, NB:2 * NB])
                return x8

            xmb_cur = load_xmb(1)
            xin_cur = new_xin8(1)
            xmb_nxt = xin_nxt = None

            # bootstrap xin(1): xhat_0 = bro, masked with x_0
            nc.vector.memset(xin_cur[0:F, 0, :], 0.0)
            nc.scalar.activation(out=xin_cur[0:F, 0, :], in_=xin_cur[0:F, 0, :],
                                 func=AF.Identity, bias=bro_sb, scale=1.0)
            nc.vector.copy_predicated(
                xin_cur[0:F, 0, :],
                xmb_cur[:, 0, NB:2 * NB].bitcast(mybir.dt.uint16),
                xmb_cur[:, 0, 0:NB])

            out_cur = oblkp.tile([128, OBLK, NB], dt.bfloat16, tag="out4", name="out0")

            def alloc_ps(t):
                # full-bank tiles (2KB/partition) so r/z/n never share a PSUM
                # bank -- bank sharing serializes readers on unrelated writers
                ps_r = psp.tile([128, 512], dt.float32, tag="ps_r", name=f"psr{t}")[:, 0:128]
                ps_z = psp.tile([128, 512], dt.float32, tag="ps_z", name=f"psz{t}")[:, 0:128]
                ps_n = psp.tile([128, 512], dt.float32, tag="ps_n", name=f"psn{t}")[:, 0:256]
                nc.tensor.matmul(ps_r, btr_sb, ind_sb[0:4, 0:128],
                                 start=True, stop=False, skip_group_check=True)
                nc.tensor.matmul(ps_z, btz_sb, ind_sb[0:4, 0:128],
                                 start=True, stop=False, skip_group_check=True)
                nc.tensor.matmul(ps_n, btn_sb, ind_sb,
                                 start=True, stop=False, skip_group_check=True)
                return ps_r, ps_z, ps_n

            ps_cur = alloc_ps(1)

            for t in range(1, n_steps + 1):
                pv, cur = (t - 1) % 2, t % 2
                s = (t - 1) % XBLK           # xin slot for step t
                so = (t - 1) % OBLK          # out slot for out index t-1

                ps_r, ps_z, ps_n = ps_cur
                ps_ro = psrop.tile([128, 512], dt.float32, tag="ps_ro",
                                   name=f"ro{t}")[:, 0:NB]

                def dst(gate, m):
                    if gate == 0:
                        return ps_r[:, m * 32:m * 32 + 32]
                    if gate == 1:
                        return ps_z[:, m * 32:m * 32 + 32]
                    return ps_n[:, 128 + m * 32:128 + m * 32 + 32]

                def gh(gate, m, c2):
                    idx = gate * 16 + m * 4 + c2
                    nc.tensor.matmul(dst(gate, m), whh_sb[:, idx, :],
                                     hb[:, c2, pv, :],
                                     start=False, stop=False,
                                     skip_group_check=True)

                def gi(gate, m, stop=False):
                    tile_i = gate * 4 + m
                    off = ps_n[:, m * 32:m * 32 + 32] if gate == 2 else dst(gate, m)
                    nc.tensor.matmul(off, wih_sb[:, tile_i, :],
                                     xin_cur[:, s, :],
                                     start=False, stop=stop,
                                     skip_group_check=True)

                # r gate first (chain-critical), then n parts, then z
                for m in range(4):
                    for c2 in range(4):
                        gh(0, m, c2)
                for m in range(4):
                    gi(0, m, stop=(m == 3))
                r_t = work.tile([128, 128], dt.bfloat16, tag="r_t", name=f"r{t}")
                nc.scalar.activation(out=r_t, in_=ps_r, func=AF.Sigmoid,
                                     scale=1.0 / WS)
                for m in range(4):
                    for c2 in range(4):
                        gh(2, m, c2)
                for m in range(4):
                    gi(2, m, stop=(m == 3))
                for m in range(4):
                    for c2 in range(4):
                        gh(1, m, c2)
                for m in range(4):
                    gi(1, m, stop=(m == 3))
                z_t = work.tile([128, 128], dt.bfloat16, tag="z_t", name=f"z{t}")
                nc.scalar.activation(out=z_t, in_=ps_z, func=AF.Sigmoid,
                                     scale=1.0 / WS)
                zh_t = work.tile([128, 128], dt.bfloat16, tag="zh", name=f"zh{t}")
                nc.gpsimd.tensor_tensor(zh_t, z_t, hb[:, :, pv, :], ALU.mult)
                # chain: tmp -> nin -> tanh -> t3 -> h'
                tmp_t = work.tile([128, 128], dt.float32, tag="tmp", name=f"tm{t}")
                nc.vector.tensor_tensor(tmp_t, ps_n[:, 128:256], r_t, ALU.mult)
                nin_t = work.tile([128, 128], dt.float32, tag="nin", name=f"ni{t}")
                nc.vector.tensor_tensor(nin_t, ps_n[:, 0:128], tmp_t, ALU.add)
                omz_t = work.tile([128, 128], dt.bfloat16, tag="omz", name=f"om{t}")
                nc.vector.tensor_scalar(omz_t, z_t, -1.0, 1.0, ALU.mult, ALU.add)
                n_t = work.tile([128, 128], dt.bfloat16, tag="n_t", name=f"n{t}")
                nc.scalar.activation(out=n_t, in_=nin_t, func=AF.Tanh,
                                     scale=1.0 / WS)
                t3_t = work.tile([128, 128], dt.bfloat16, tag="t3", name=f"t3{t}")
                nc.vector.tensor_tensor(t3_t, n_t, omz_t, ALU.mult)
                # next step's bias matmuls: emitted here so they sit in the
                # PE stream's idle window (before ro), not ahead of gh-r
                if t < n_steps:
                    ps_nxt = alloc_ps(t + 1)
                # readout: Wro^T zh + Wro^T t3  (= Wro^T h_new)
                for k in range(4):
                    nc.tensor.matmul(ps_ro, wro_sb[:, k, :],
                                     zh_t[:, k * 32:(k + 1) * 32],
                                     start=(k == 0), stop=False,
                                     skip_group_check=True)
                for k in range(4):
                    nc.tensor.matmul(ps_ro, wro_sb[:, k, :],
                                     t3_t[:, k * 32:(k + 1) * 32],
                                     start=False, stop=(k == 3),
                                     skip_group_check=True)
                # h_new
                nc.vector.tensor_tensor(hb[:, :, cur, :], t3_t, zh_t, ALU.add)

                # stage output for out index t-1
                nc.scalar.activation(out=out_cur[:, so, :], in_=ps_ro, func=AF.Copy)
                if so == OBLK - 1 or t == n_steps:
                    nc.sync.dma_start(out=op_out[:, t - 1 - so:t, :],
                                      in_=out_cur[:, 0:so + 1, :])
                    if t < n_steps:
                        out_cur = oblkp.tile([128, OBLK, NB], dt.bfloat16,
                                             tag="out4", name=f"out{t}")

                # prefetch next x/mask block
                if s == 2:
                    t0n = t - s + XBLK
                    if t0n <= n_steps:
                        xmb_nxt = load_xmb(t0n)
                        xin_nxt = new_xin8(t0n)

                # x_in(t+1) feedback
                if t < n_steps:
                    s2 = t % XBLK
                    if s2 == 0:
                        xmb_cur, xin_cur = xmb_nxt, xin_nxt
                    nc.vector.tensor_scalar_add(xin_cur[0:F, s2, :],
                                                ps_ro[0:F, :], bro_sb)
                    nc.vector.copy_predicated(
                        xin_cur[0:F, s2, :],
                        xmb_cur[:, s2, NB:2 * NB].bitcast(mybir.dt.uint16),
                        xmb_cur[:, s2, 0:NB])
                    ps_cur = ps_nxt

    _legalize_multiwait(nc)
    return nc


_NC_CACHE = {}


def _get_nc(n_steps):
    if n_steps not in _NC_CACHE:
        _NC_CACHE[n_steps] = build_nc(n_steps)
    return _NC_CACHE[n_steps]


def _prep_core_inputs(x2d, m2d, Wih, Whh, bih, bhh, Wro, bro, Wout_half, n_steps):
    """Per-core input map. x2d/m2d: [NB, S_loc, F] float32/bool already
    direction-ordered (time-reversed for backward cores)."""
    xt = x2d[:, :n_steps].transpose(2, 1, 0)          # [F, t, NB]
    mt = m2d[:, :n_steps].transpose(2, 1, 0).astype(np.float32)
    xmv = np.concatenate([xt, mt], axis=2).astype(BF16)  # [F, t, 2*NB]

    WT = np.ascontiguousarray(Whh.T).astype(np.float32) * WS  # [512, 1536]
    whh_t = np.empty((128, 48, 128), np.float32)
    for g in range(3):
        for m in range(4):
            for c2 in range(4):
                idx = g * 16 + m * 4 + c2
                whh_t[:, idx, :] = WT[128 * c2:128 * (c2 + 1),
                                      512 * g + 128 * m:512 * g + 128 * (m + 1)]
    WI = np.ascontiguousarray(Wih.T).astype(np.float32) * WS  # [128, 1536]
    wih_t = WI.reshape(128, 12, 128)

    wro_f = Wro.T.reshape(NFOLD, 128, F)               # [4,128,64]
    wout_f = Wout_half.T.reshape(NFOLD, 128, F)
    wro_t = np.concatenate([wro_f, wout_f], axis=2).transpose(1, 0, 2)  # [128,4,128]

    bsum = (bih + bhh).astype(np.float32) * WS
    btr = np.empty((4, 128), np.float32)
    btz = np.empty((4, 128), np.float32)
    for k in range(4):
        btr[k] = bsum[128 * k:128 * (k + 1)]                   # r folds
        btz[k] = bsum[512 + 128 * k:512 + 128 * (k + 1)]       # z folds
    btn = np.empty((8, 128), np.float32)
    for k in range(4):
        btn[k] = bih.astype(np.float32)[1024 + 128 * k:1024 + 128 * (k + 1)] * WS
        btn[4 + k] = bhh.astype(np.float32)[1024 + 128 * k:1024 + 128 * (k + 1)] * WS
    ind = np.zeros((8, 256), np.float32)
    for k in range(4):
        ind[k, 32 * k:32 * (k + 1)] = 1.0
        ind[4 + k, 128 + 32 * k:128 + 32 * (k + 1)] = 1.0

    return {
        "xm": xmv,
        "whh": whh_t.astype(BF16).reshape(128, 48 * 128),
        "wih": wih_t.astype(BF16).reshape(128, 12 * 128),
        "wro": np.ascontiguousarray(wro_t).astype(BF16).reshape(128, 4 * 128),
        "btr": btr.astype(BF16), "btz": btz.astype(BF16),
        "btn": btn.astype(BF16),
        "indt": ind.astype(BF16),
        "bro": bro.reshape(F, 1).astype(np.float32),
    }


def run_device(inputs, s_len=S, trace=False):
    n_steps = s_len - 1
    nc = _get_nc(n_steps)

    x2d = np.asarray(inputs["x"], np.float32).reshape(B, S, F)[:, :s_len]
    m2d = np.asarray(inputs["mask"]).reshape(B, S, F)[:, :s_len]

    in_maps = []
    for core in range(8):
        g = core % 4
        bsl = slice(NB * g, NB * (g + 1))
        if core < 4:
            im = _prep_core_inputs(
                x2d[bsl], m2d[bsl], inputs["Wih_f"], inputs["Whh_f"],
                inputs["bih_f"], inputs["bhh_f"], inputs["Wro_f"], inputs["bro_f"],
                np.asarray(inputs["Wout"])[:, :H], n_steps,
            )
        else:
            im = _prep_core_inputs(
                x2d[bsl, ::-1], m2d[bsl, ::-1], inputs["Wih_b"], inputs["Whh_b"],
                inputs["bih_b"], inputs["bhh_b"], inputs["Wro_b"], inputs["bro_b"],
                np.asarray(inputs["Wout"])[:, H:], n_steps,
            )
        in_maps.append(im)

    return run_bass_kernel_spmd(nc, in_maps, core_ids=list(range(8)), trace=trace)


def assemble(inputs, res, s_len=S):
    """Host-side gather: combine per-core outputs into full reference outputs."""
    n_steps = s_len - 1
    bro_f = np.asarray(inputs["bro_f"], np.float32)
    bro_b = np.asarray(inputs["bro_b"], np.float32)
    bout = np.asarray(inputs["bout"], np.float32)

    xh_f = np.empty((B, s_len, F), np.float32)
    xh_b = np.empty((B, s_len, F), np.float32)
    x_hat = np.empty((B, s_len, F), np.float32)

    for g in range(4):
        bsl = slice(NB * g, NB * (g + 1))
        rf = np.asarray(res.results[g]["op"], np.float32)       # [128, t, NB]
        rb = np.asarray(res.results[g + 4]["op"], np.float32)
        xh_f[bsl, 1:] = rf[:F].transpose(2, 1, 0) + bro_f
        xh_f[bsl, 0] = bro_f
        xh_b[bsl, :n_steps] = rb[:F].transpose(2, 1, 0)[:, ::-1] + bro_b
        xh_b[bsl, n_steps] = bro_b
        pf = rf[F:].transpose(2, 1, 0)
        pb = rb[F:].transpose(2, 1, 0)[:, ::-1]
        x_hat[bsl, 1:] = pf
        x_hat[bsl, 0] = 0.0
        x_hat[bsl, :n_steps] += pb
        x_hat[bsl] += bout

    return (
        x_hat.reshape(B, s_len, N, C),
        xh_f.reshape(B, s_len, N, C),
        xh_b.reshape(B, s_len, N, C),
    )


def kernel(**inputs):
    res = run_device(inputs, s_len=S)
    return assemble(inputs, res, s_len=S)
